# revision 19
# baseline (speedup 1.0000x reference)
"""Multi-head attention Trainium2 kernel (8 NeuronCores, SPMD).

Problem: B=4, S=2048, D_MODEL=1024, H=16, DIM=64 (nn_MultiHeadAttn).
Sharding: core c handles (batch b = c//2, query-row chunk c%2 of 1024).
Each core computes all 16 heads for its 1024 query rows against the full
2048 keys of its batch, then its rows of the output projection.

v2: the K and V projections are folded away on the host:
  - scores = qh.kh^T == q (Wq^T Wk) k^T (+ a per-query constant from bk
    that softmax ignores, dropped).  Only the Q side is projected, by
    M = Wq^T Wk with bias c = Wk^T bq; raw k^T is the scores lhsT.
  - out = attn_norm @ (v Wv^T + bv) @ Wo^T + bo
        == attn_norm @ v @ (Wo')^T + bo'  with Wo' = Wo . blockdiag(Wv),
    bo' = bo + Wo @ tile(bv) (softmax rows sum to 1): AV runs on raw v.

v3: everything PSUM-resident is a single bank wide (512 f32 columns) so
the attention pipeline is deep despite PSUM's 8-bank limit: the scores
ring holds ~2.5 key-chunks in flight (PE never waits on exp latency) and
the av ring covers pair transitions.  Queries processed in two blocks of
512 per pair.

Device dataflow per head-pair (8 pairs), per query-block qb (2 of 512):
  - qm^T[e2, q] via block-diagonal [128,128] M-weights + bias (ScalarE).
  - scores^T[k, qb] = (k2 half).T @ qm^T per 128-key chunk; head A on PE
    row-groups 0-1, head B on 2-3 (tile_position) so the pair packs.
  - exp with the 1/8 scale folded in (no max subtraction: |scores| < ~20,
    softmax shift-invariant).  Head A on ScalarE (spline exp), head B on
    VectorE (custom 8-stage DVE op: deg-3 p(x)~=exp(x/32) squared twice).
  - out_h^T[e, qb] (+ sum-of-exp in row 64) = (v|1).T @ attn^T
    accumulated over the 16 key chunks in one PSUM bank per half.
  - normalize: reciprocal straight off the PSUM sum row (DVE custom op),
    gpsimd partition-broadcast, multiply; head B staged and DMA'd into
    hidden partitions 64:128 (engines are partition-aligned).
  - out^T[o, q] = Wo'^T-tiles.T @ hidden^T accumulated over e-tiles.
"""

import sys

if "/opt/trn_rl_repo" not in sys.path:
    sys.path.insert(0, "/opt/trn_rl_repo")

import numpy as np
from contextlib import ExitStack

N_CORES = 8
B, S, D = 4, 2048, 1024
H, DIM = 16, 64
SQ = 1024          # query rows per core
QB = 512           # query block (one PSUM bank of f32)
NQB = SQ // QB
NPAIR = 8          # head pairs
NKC = S // 128     # key chunks of 128
VAW = 130          # width of augmented V block (64 + 1) * 2

# deg-3 minimax fit of exp(x/32) on |x|<=20; kernel computes p(x)^4=exp(x/8).
EXPC3 = 4.98779571e-06
EXPC2 = 5.03750782e-04
EXPC1 = 3.13034249e-02
EXPC0 = 9.99313241e-01

_cache = {}


def _register_exp_op():
    """Register the custom DVE exp op (deg-3 Horner + 2 squarings, 8 ALU
    stages) in concourse's custom-DVE registry; the per-NEFF uop table is
    generated from dve_ops.OPS at compile time."""
    if "exp_op" in _cache:
        return _cache["exp_op"]
    from concourse import dve_ops
    from concourse.dve_spec import (
        Spec, Src0, C0, C1, C2, C3, sq, lower, _spill_c3_to_src1,
    )
    from concourse.dve_uop import DveOpSpec
    from concourse.dve_table_gen import dve_ver_for

    name = "EXP_POLY4_ANT"
    for op in dve_ops.OPS:
        if op.name == name:
            _cache["exp_op"] = op
            return op

    def _ref(in0, in1, s0, s1, imm2):
        p = ((s0 * in0 + s1) * in0 + imm2) * in0 + in1
        return (p * p) * (p * p)

    body = sq(sq(((C0 * Src0 + C1) * Src0 + C2) * Src0 + C3))
    spec = Spec(body=_spill_c3_to_src1(body), reference=_ref)
    dve_ops._SUB_OPCODE_FOR_NAME[name] = dve_ops._CUSTOM_DVE_ROW_BASE + len(dve_ops.OPS)
    shas = {}
    for ver in ("v3", "v4"):
        try:
            tmp = DveOpSpec(name=name, opcode=dve_ops.get_dve_sub_opcode(name),
                            uops=lower(spec, ver=ver), rd1_en=True)
            shas[ver] = tmp.sha(ver)
        except Exception:
            pass
    op = dve_ops.DveOp(name, spec, subdim=False, uops_sha=shas)
    dve_ops.OPS.append(op)
    dve_ops.CUSTOM_DVE_SPECS[name] = spec
    _cache["exp_op"] = op
    return op


def _build_program():
    from concourse import bacc, mybir, tile

    exp_op = _register_exp_op()

    f32 = mybir.dt.float32
    bf16 = mybir.dt.bfloat16
    Exp = mybir.ActivationFunctionType.Exp
    Ident = mybir.ActivationFunctionType.Identity

    nc = bacc.Bacc("TRN2", target_bir_lowering=False, debug=False)

    qT = nc.dram_tensor("qT", [D, SQ], bf16, kind="ExternalInput")
    kT = nc.dram_tensor("kT", [D, S], bf16, kind="ExternalInput")
    vaT = nc.dram_tensor("vaT", [NPAIR, S, VAW], bf16, kind="ExternalInput")
    m2 = nc.dram_tensor("m2", [128, 128], bf16, kind="ExternalInput")
    qc2 = nc.dram_tensor("qc2", [128, 1], f32, kind="ExternalInput")
    woT = nc.dram_tensor("woT", [D, D], bf16, kind="ExternalInput")
    bod = nc.dram_tensor("bod", [D, 1], f32, kind="ExternalInput")
    outT = nc.dram_tensor("outT", [D, SQ], f32, kind="ExternalOutput")

    with tile.TileContext(nc) as tc:
        with ExitStack() as ctx:
            ep = ctx.enter_context
            consts = ep(tc.tile_pool(name="consts", bufs=1))
            raw = ep(tc.tile_pool(name="raw", bufs=1))
            projq = ep(tc.tile_pool(name="projq", bufs=1))
            attn_p = ep(tc.tile_pool(name="attn", bufs=6))
            norm_p = ep(tc.tile_pool(name="norm", bufs=3))
            hid_p = ep(tc.tile_pool(name="hid", bufs=1))
            outs_p = ep(tc.tile_pool(name="outs", bufs=2))
            sc_ps = ep(tc.tile_pool(name="scps", bufs=2, space="PSUM"))
            ava_ps = ep(tc.tile_pool(name="avaps", bufs=2, space="PSUM"))
            avb_ps = ep(tc.tile_pool(name="avbps", bufs=2, space="PSUM"))

            # ---- constants ----
            woT_s = consts.tile([128, 8, D], bf16, tag="woT")
            nc.sync.dma_start(woT_s[:], woT.rearrange("(et p) o -> p et o", p=128))
            bo_s = consts.tile([128, 8], f32, tag="bo")
            nc.sync.dma_start(bo_s[:], bod.rearrange("(ot p) one -> p (ot one)", p=128))
            m2_s = consts.tile([128, 128], bf16, tag="m2")
            nc.sync.dma_start(m2_s[:], m2[:, :])
            qc2_s = consts.tile([128, 1], f32, tag="qc2")
            nc.sync.dma_start(qc2_s[:], qc2[:, :])
            c3t = consts.tile([128, 1], f32, tag="c3t")
            nc.vector.memset(c3t[:], EXPC0)

            hidden = hid_p.tile([128, 8, SQ], bf16, tag="hidden")

            # ---- all raw inputs resident up front (no DMA waits at pair
            # transitions); per-pair slices issued as separate DMAs so
            # they spread across queues.
            q2_all = raw.tile([128, NPAIR, SQ], bf16, tag="q2")
            k2_all = raw.tile([128, NPAIR, S], bf16, tag="k2")
            va_all = raw.tile([128, NPAIR, NKC, VAW], bf16, tag="va")
            for pair in range(NPAIR):
                rows = slice(pair * 128, (pair + 1) * 128)
                nc.sync.dma_start(q2_all[:, pair, :], qT[rows, :])
                nc.sync.dma_start(k2_all[:, pair, :], kT[rows, :])
                nc.sync.dma_start(va_all[:, pair, :, :], vaT[pair].rearrange(
                    "(kc p) w -> p kc w", p=128))

            # ---- PE warm-up: ~5us of back-to-back matmuls flips the HAM
            # clock gate to 8/8 (2.4 GHz) before real work arrives.
            warm = sc_ps.tile([128, 2 * QB], f32, tag="sc")
            for _ in range(12):
                nc.tensor.matmul(warm[:, 0:QB], woT_s[:, 0, 0:128],
                                 woT_s[:, 1, 0:512], start=True, stop=True)

            # ---- Q projections for all pairs (bias-add on ScalarE) ----
            qh_all = projq.tile([128, NPAIR, SQ], bf16, tag="qh")
            for pair in range(NPAIR):
                ps = sc_ps.tile([128, 2 * QB], f32, tag="sc")
                for j in range(NQB):
                    nc.tensor.matmul(ps[:, j * QB:(j + 1) * QB], m2_s[:],
                                     q2_all[:, pair, j * QB:(j + 1) * QB],
                                     start=True, stop=True)
                nc.scalar.activation(qh_all[:, pair, :], ps[:], Ident,
                                     bias=qc2_s[:])

            for pair in range(NPAIR):

                # ---- attention, one query-block at a time ----
                # One [128, 2*QB] PSUM tile per key chunk holds head A's
                # scores in columns 0:QB and head B's in QB:2QB, so a
                # single wide exp instruction (alternating ScalarE/DVE per
                # chunk) covers the pair.  AV matmuls for chunk kc-1 are
                # emitted after chunk kc's scores+exp (software pipeline).
                for qb in range(NQB):
                    qs = slice(qb * QB, (qb + 1) * QB)
                    avA = ava_ps.tile([65, QB], f32, tag="ava")
                    avB = avb_ps.tile([65, QB], f32, tag="avb")
                    ats = [None] * NKC

                    def do_av(kc):
                        first, last = kc == 0, kc == NKC - 1
                        at = ats[kc]
                        nc.tensor.matmul(avA[:], va_all[:, pair, kc, 0:65],
                                         at[:, 0:QB], start=first, stop=last)
                        nc.tensor.matmul(avB[:], va_all[:, pair, kc, 65:130],
                                         at[:, QB:2 * QB], start=first,
                                         stop=last)

                    for kc in range(NKC):
                        ks = slice(kc * 128, (kc + 1) * 128)
                        sc = sc_ps.tile([128, 2 * QB], f32, tag="sc")
                        nc.tensor.matmul(sc[:, 0:QB], k2_all[0:64, pair, ks],
                                         qh_all[0:64, pair, qs], start=True, stop=True,
                                         tile_position=(0, 0))
                        nc.tensor.matmul(sc[:, QB:2 * QB], k2_all[64:128, pair, ks],
                                         qh_all[64:128, pair, qs], start=True, stop=True,
                                         tile_position=(64, 0))
                        at = attn_p.tile([128, 2 * QB], bf16, tag="attn")
                        ats[kc] = at
                        if kc % 2 == 0:
                            nc.scalar.activation(at[:], sc[:], Exp, scale=0.125)
                        else:
                            nc.vector._custom_dve(
                                exp_op, out=at[:], in0=sc[:], in1=c3t[:],
                                s0=EXPC3, s1=EXPC2, imm2=EXPC1)
                        if kc >= 1:
                            do_av(kc - 1)
                    do_av(NKC - 1)

                    # ---- normalize: hidden^T[e,q] = av[e,q] / av[64,q] ----
                    for half, av in ((0, avA), (1, avB)):
                        rb = norm_p.tile([65, QB], f32, tag="rb")
                        nc.scalar.copy(rb[64:65, :], av[64:65, :])
                        sums = norm_p.tile([1, QB], f32, tag="sums")
                        nc.sync.dma_start(sums[:], rb[64:65, :])
                        recip = norm_p.tile([1, QB], f32, tag="recip")
                        nc.vector.reciprocal_approx_fast(recip[:], sums[:])
                        nc.gpsimd.partition_broadcast(rb[0:64, :], recip[:])
                        if half == 0:
                            nc.vector.tensor_tensor(
                                hidden[0:64, pair, qs], av[0:64, :],
                                rb[0:64, :], op=mybir.AluOpType.mult)
                        else:
                            stg = norm_p.tile([64, QB], bf16, tag="stg")
                            nc.vector.tensor_tensor(
                                stg[:], av[0:64, :], rb[0:64, :],
                                op=mybir.AluOpType.mult)
                            nc.sync.dma_start(hidden[64:128, pair, qs], stg[:])

            # ---- output projection: out^T[o, q] ----
            for ot in range(8):
                o_s = outs_p.tile([128, SQ], f32, tag="outs")
                pso = sc_ps.tile([128, 2 * QB], f32, tag="sc")
                for j in range(NQB):
                    qs = slice(j * QB, (j + 1) * QB)
                    for et in range(8):
                        nc.tensor.matmul(pso[:, j * QB:(j + 1) * QB],
                                         woT_s[:, et, ot * 128:(ot + 1) * 128],
                                         hidden[:, et, qs],
                                         start=(et == 0), stop=(et == 7))
                nc.scalar.activation(o_s[:], pso[:], Ident,
                                     bias=bo_s[:, ot:ot + 1])
                nc.sync.dma_start(outT[ot * 128:(ot + 1) * 128, :], o_s[:])

    nc.compile()
    return nc


def _get_nc():
    if "nc" not in _cache:
        _cache["nc"] = _build_program()
    return _cache["nc"]


def _prep_consts(Wq, bq, Wk, bk, Wv, bv, Wo, bo):
    f = np.float32

    def blockdiag2(W):
        out = np.zeros((128, 128), f)
        out[:64, :64] = W.T
        out[64:, 64:] = W.T
        return out

    # Q-side fold: ps[e,s] = sum_d m2[d,e] q2[d,s] needs m2[d,e] =
    # (Wq.T @ Wk)[d,e]; blockdiag2 embeds W.T, so pass W = Wk.T @ Wq.
    M = (Wk.T @ Wq).astype(f)
    c = (Wk.T @ bq).astype(f)
    # Wo' absorbs Wv; bo' absorbs bv (softmax rows sum to 1).
    WoH = Wo.reshape(D, H, DIM)
    WoP = np.einsum('ohe,ed->ohd', WoH, Wv).reshape(D, D).astype(f)
    boP = (bo + np.einsum('ohe,e->o', WoH, bv)).astype(f)
    import ml_dtypes
    b16 = ml_dtypes.bfloat16
    return {
        "m2": blockdiag2(M).astype(b16),
        "qc2": np.tile(c, 2)[:, None].copy(),
        "woT": np.ascontiguousarray(WoP.T).astype(b16),
        "bod": boP[:, None].copy(),
    }


def _prep_va(v):
    """v [S, D] f32 -> augmented natural-layout [NPAIR, S, VAW] bf16:
    per pair p, columns [v_headA(64) | 1 | v_headB(64) | 1]."""
    import ml_dtypes
    b16 = ml_dtypes.bfloat16
    vh = v.reshape(S, H, DIM)
    va = np.ones((NPAIR, S, VAW), np.float32)
    for p in range(NPAIR):
        va[p, :, 0:64] = vh[:, 2 * p, :]
        va[p, :, 65:129] = vh[:, 2 * p + 1, :]
    return np.ascontiguousarray(va).astype(b16)


def kernel(q, k, v, Wq, bq, Wk, bk, Wv, bv, Wo, bo, _trace=False):
    import ml_dtypes
    b16 = ml_dtypes.bfloat16
    q = np.asarray(q, np.float32)
    k = np.asarray(k, np.float32)
    v = np.asarray(v, np.float32)
    consts = _prep_consts(
        np.asarray(Wq, np.float32), np.asarray(bq, np.float32),
        np.asarray(Wk, np.float32), np.asarray(bk, np.float32),
        np.asarray(Wv, np.float32), np.asarray(bv, np.float32),
        np.asarray(Wo, np.float32), np.asarray(bo, np.float32))

    in_maps = []
    va_cache = {}
    for c in range(N_CORES):
        b, chunk = c // 2, c % 2
        m = dict(consts)
        m["qT"] = np.ascontiguousarray(
            q[b, chunk * SQ:(chunk + 1) * SQ, :].T).astype(b16)
        m["kT"] = np.ascontiguousarray(k[b].T).astype(b16)
        if b not in va_cache:
            va_cache[b] = _prep_va(v[b])
        m["vaT"] = va_cache[b]
        in_maps.append(m)

    nc = _get_nc()
    from concourse.bass_utils import run_bass_kernel_spmd
    res = run_bass_kernel_spmd(nc, in_maps, core_ids=list(range(N_CORES)),
                               trace=_trace)
    if _trace:
        kernel.last_results = res

    out = np.empty((B, S, D), np.float32)
    for c in range(N_CORES):
        b, chunk = c // 2, c % 2
        out[b, chunk * SQ:(chunk + 1) * SQ, :] = res.results[c]["outT"].T
    return out
